# revision 39
# baseline (speedup 1.0000x reference)
"""CSPN (convolutional spatial propagation network) Trainium2 Bass kernel.

Problem: 16 iterations of
    d <- blend(max_c[ box3x3(g_c * d) / box3x3(g_c) ], sparse)
with g = |guidance| [B=8, C=8, H=256, W=512], d = depth [B,1,H,W].

Strategy (8 NeuronCores, pure batch data-parallel — one image per core):
- H=256 split into two partition blocks of 128 (hb0 = rows 0..127 natural,
  hb1 = rows 128..255 stored ROW-REVERSED via a host-side flip). With this
  layout both inter-block boundary rows live on partition 127.
- box3x3(e): per (hb, 4-channel) unit, 12 PSUM-accumulated matmuls with a
  tridiagonal ones stationary T and the rhs shifted -1/0/+1 along W.
- Partition 127 (the boundary rows, which need the OTHER block's boundary
  row) is computed redundantly in a 16-partition side domain from staged g
  rows 127/126 and per-iteration staged d rows 127/126, pre-scaled by a
  staged ivm row, and injected into sraw by one early plain DMA; the PSUM
  evacuation (ACT) and the ivm scale (Pool/DVE) only touch partitions
  0..126, so the boundary never sits on the iteration-critical tail.
- The max over 8 gate channels is a pairwise DVE tree; d = dmax + sparse.
- Dummy matmuls bridge the inter-iteration gap to hold the tensor
  engine's p-state ramp.
- Everything in the loop is fp16 (DVE 2x mode) with fp32 PSUM accumulation.
"""

import sys
import numpy as np

sys.path.insert(0, "/opt/trn_rl_repo")

B, C, H, W = 8, 8, 256, 512
PB = 128          # partitions per h-block
WP = W + 2        # padded width (zero col at each end)
NUM_ITERS = 16
N_CORES = 8
HC = 4            # channels per unit (PSUM bank group = 4 banks)

_BUILT = None

# scale-engine per unit [(0,0),(0,1),(1,0),(1,1)]: "cce" = Pool DMA mult
MUL_ENG = ["dve", "dve", "dve", "dve"]
# run l2/l3 of each hb's max tree on the (mostly idle) Pool engine?
TREE_POOL = [False, False]
N_DUMMY = 28


def _bcast_c(ap, nch, bass):
    """Insert a broadcast (step 0) channel dim after the partition dim."""
    aps = [list(x) for x in ap.ap]
    assert len(aps) == 2
    return bass.AP(tensor=ap.tensor, offset=ap.offset,
                   ap=[aps[0], [0, nch], aps[1]])


def _build(num_iters=NUM_ITERS, variant="full", n_dummy=None):
    global MUL_ENG
    if n_dummy is None:
        n_dummy = N_DUMMY
    import concourse.bacc as bacc
    import concourse.bass as bass
    import concourse.tile as tile
    from concourse import mybir

    f16, f32 = mybir.dt.float16, mybir.dt.float32
    Alu = mybir.AluOpType

    # scale-engine for units [(0,0),(0,1),(1,0),(1,1)]: cce = Pool DMA
    mul_eng = MUL_ENG

    nc = bacc.Bacc("TRN2", target_bir_lowering=False, debug=False)

    g_d = nc.dram_tensor("guidance", [C, 2, PB, W], f32, kind="ExternalInput")
    b_d = nc.dram_tensor("blur", [2, PB, W], f32, kind="ExternalInput")
    s_d = nc.dram_tensor("sparse", [2, PB, W], f32, kind="ExternalInput")
    t_d = nc.dram_tensor("tband", [PB, PB], f16, kind="ExternalInput")
    o_d = nc.dram_tensor("out", [2, PB, W], f32, kind="ExternalOutput")

    UNITS = [(0, 0), (0, 1), (1, 0), (1, 1)]

    with tile.TileContext(nc) as tc, nc.allow_low_precision(
        "fp16 by design: diffusion is a convex combination, error stays ~eps"
    ):
        import contextlib
        ctx = contextlib.ExitStack()
        with ctx:
            perm = ctx.enter_context(tc.tile_pool(name="perm", bufs=1))
            epool = ctx.enter_context(tc.tile_pool(name="ep", bufs=1))
            g_t = perm.tile([PB, 2, C, WP], f16)
            ivm_t = perm.tile([PB, 2, C, W], f16)   # (1/wsum)*(1-mask)
            sraw = [perm.tile([PB, 2, C, W], f16, name=f"sraw{i}")
                    for i in range(2)]
            l1_t = perm.tile([PB, 2, 4, W], f16)
            l2_t = perm.tile([PB, 2, 2, W], f16)
            dmax_t = perm.tile([PB, 2, W], f16)
            d_t = perm.tile([PB, 2, WP], f16)
            sp_t = perm.tile([PB, 2, W], f16)
            T_t = perm.tile([PB, PB], f16)
            # side-domain gate rows: slot 0 = own block row 127, slot 1 =
            # other block row 127, slot 2 = own block row 126
            g01s_t = perm.tile([2 * C, 3, WP], f16)
            ivm0s_t = perm.tile([2 * C, W], f16)  # ivm boundary row

            nc.sync.dma_start(T_t[:], t_d[:])

            psum = ctx.enter_context(
                tc.tile_pool(name="ps", bufs=1, space="PSUM"))

            def box_unit(hb, cg, src, ps):
                """12 tridiagonal matmuls into ps (colsum x rowsum shifts)."""
                c0 = cg * HC
                for c in range(HC):
                    for s in range(3):
                        nc.tensor.matmul(ps[:, c, :], T_t[:],
                                         src[:, hb, c0 + c, s:s + W],
                                         start=(s == 0), stop=(s == 2))

            def rep_row(row, hb):
                """AP: d_t[row, hb, :] replicated C times (partition row)."""
                src = d_t[row:row + 1, hb, :]
                aps = [list(x) for x in src.ap]
                return bass.AP(tensor=src.tensor, offset=src.offset,
                               ap=[aps[0], [0, C], aps[1]])

            def rowsum3(out, src):
                """out[:, s, w] = src[:, s, w] + src[:, s, w+1] + src[:, s,
                w+2] over the padded last dim (3-slot side tiles)."""
                nc.vector.tensor_add(out[:], src[:, :, 0:W],
                                     src[:, :, 2:2 + W])
                nc.vector.tensor_add(out[:], out[:], src[:, :, 1:1 + W])

            def side_row0(d01s, eb01, bs01, row0, sr):
                """Compute the full scaled box row for partition 127 of both
                blocks in a [16, W] side domain and inject it into sraw:
                row0[hb*8+c] = (rs3(e[127,hb,c]) + rs3(e[126,hb,c])
                                + rs3(e[127,1-hb,c])) * ivm[127,hb,c].
                All combines stay at the same base partition (slots are
                free-dim)."""
                for hb in range(2):
                    nc.sync.dma_start(d01s[hb * C:(hb + 1) * C, 0, :],
                                      rep_row(PB - 1, hb))
                    nc.sync.dma_start(d01s[hb * C:(hb + 1) * C, 1, :],
                                      rep_row(PB - 1, 1 - hb))
                    nc.sync.dma_start(d01s[hb * C:(hb + 1) * C, 2, :],
                                      rep_row(PB - 2, hb))
                nc.vector.tensor_mul(eb01[:], g01s_t[:], d01s[:])
                rowsum3(bs01, eb01)
                nc.vector.tensor_add(row0[:], bs01[:, 0, :], bs01[:, 1, :])
                nc.vector.tensor_add(row0[:], row0[:], bs01[:, 2, :])
                nc.vector.tensor_mul(row0[:], row0[:], ivm0s_t[:])
                nc.sync.dma_start(sr[PB - 1:PB, :, :, :], row0[:])

            def evac_unit(hb, cg, ps, sr):
                c0 = cg * HC
                nc.scalar.copy(sr[0:PB - 1, hb, c0:c0 + HC, :],
                               ps[0:PB - 1, :, :])

            def mul_unit(hb, cg, uidx, sr):
                c0 = cg * HC
                if mul_eng[uidx] == "pool":
                    nc.gpsimd.tensor_mul(sr[0:PB - 1, hb, c0:c0 + HC, :],
                                         sr[0:PB - 1, hb, c0:c0 + HC, :],
                                         ivm_t[0:PB - 1, hb, c0:c0 + HC, :])
                elif mul_eng[uidx] == "dvelate":
                    # same product, but with a bypassed per-partition scalar
                    # read of dmax[:, 0] so this op CANNOT be scheduled ahead
                    # of tree(0) — keeps the DVE queue from blocking the
                    # iteration-critical tail on the last unit's evac.
                    nc.vector.scalar_tensor_tensor(
                        sr[0:PB - 1, hb, c0:c0 + HC, :],
                        sr[0:PB - 1, hb, c0:c0 + HC, :],
                        dmax_t[0:PB - 1, 0, 0:1],
                        ivm_t[0:PB - 1, hb, c0:c0 + HC, :],
                        Alu.bypass, Alu.mult)
                else:
                    nc.vector.tensor_mul(sr[0:PB - 1, hb, c0:c0 + HC, :],
                                         sr[0:PB - 1, hb, c0:c0 + HC, :],
                                         ivm_t[0:PB - 1, hb, c0:c0 + HC, :])

            def tree(hb, sr):
                mid = nc.gpsimd if TREE_POOL[hb] else nc.vector
                nc.vector.tensor_max(l1_t[:, hb], sr[:, hb, 0:4, :],
                                     sr[:, hb, 4:8, :])
                mid.tensor_max(l2_t[:, hb], l1_t[:, hb, 0:2, :],
                               l1_t[:, hb, 2:4, :])
                mid.tensor_max(dmax_t[:, hb], l2_t[:, hb, 0, :],
                               l2_t[:, hb, 1, :])
                nc.vector.tensor_add(d_t[:, hb, 1:1 + W], dmax_t[:, hb],
                                     sp_t[:, hb])

            def front(hb, cg, e):
                """e[:, hb, chans, :] = g * d (full padded width; g pads are
                zero so e pads stay zero)."""
                c0 = cg * HC
                dbc = _bcast_c(d_t[:, hb], HC, bass)
                nc.vector.tensor_mul(e[:, hb, c0:c0 + HC, :],
                                     g_t[:, hb, c0:c0 + HC, :], dbc)

            # ---------------- pre-loop ----------------
            with tc.tile_pool(name="trans", bufs=1) as trans:
                b32 = trans.tile([PB, 2, W], f32, tag="b32")
                s32 = trans.tile([PB, 2, W], f32, tag="s32")
                m_t = trans.tile([PB, 2, W], f16, tag="m16")
                im_t = trans.tile([PB, 2, W], f16, tag="im16")
                b16 = trans.tile([PB, 2, W], f16, tag="b16")

                nc.sync.dma_start(b32[:], b_d[:].rearrange("h p w -> p h w"))
                nc.sync.dma_start(s32[:], s_d[:].rearrange("h p w -> p h w"))

                nc.vector.tensor_copy(sp_t[:], s32[:])
                nc.scalar.sign(m_t[:], s32[:])  # sparse>=0 -> mask in {0,1}
                nc.vector.tensor_scalar(im_t[:], m_t[:], -1.0, 1.0,
                                        Alu.mult, Alu.add)
                nc.vector.tensor_copy(b16[:], b32[:])
                nc.vector.memset(d_t[:], 0.0)
                nc.vector.tensor_mul(d_t[:, :, 1:1 + W], im_t[:], b16[:])
                nc.vector.tensor_add(d_t[:, :, 1:1 + W],
                                     d_t[:, :, 1:1 + W], sp_t[:])

                # guidance |.| -> fp16 padded layout
                for hb in range(2):
                    gf = trans.tile([PB, C, W], f32, tag="big0")
                    nc.sync.dma_start(
                        gf[:], g_d[:, hb].rearrange("c p w -> p c w"))
                    nc.vector.memset(g_t[:, hb, :, 0:1], 0.0)
                    nc.vector.memset(g_t[:, hb, :, WP - 1:WP], 0.0)
                    nc.scalar.activation(g_t[:, hb, :, 1:1 + W], gf[:],
                                         mybir.ActivationFunctionType.Abs)

                # stage boundary-row gates (3-slot: own 127, other 127,
                # own 126)
                for hb in range(2):
                    nc.sync.dma_start(g01s_t[hb * C:(hb + 1) * C, 0, :],
                                      g_t[PB - 1:PB, hb, :, :])
                    nc.sync.dma_start(g01s_t[hb * C:(hb + 1) * C, 1, :],
                                      g_t[PB - 1:PB, 1 - hb, :, :])
                    nc.sync.dma_start(g01s_t[hb * C:(hb + 1) * C, 2, :],
                                      g_t[PB - 2:PB - 1, hb, :, :])

                # wsum -> ivm = (1/wsum)*(1-mask), via the same box pipeline
                bsw = trans.tile([2 * C, 3, W], f16, tag="bsw")
                wrow0 = trans.tile([2 * C, W], f16, tag="wrow0")
                rowsum3(bsw, g01s_t)
                nc.vector.tensor_add(wrow0[:], bsw[:, 0, :], bsw[:, 1, :])
                nc.vector.tensor_add(wrow0[:], wrow0[:], bsw[:, 2, :])
                nc.sync.dma_start(sraw[0][PB - 1:PB, :, :, :], wrow0[:])
                for ui, (hb, cg) in enumerate(UNITS):
                    ps = psum.tile([PB, HC, W], f32, tag=f"ps{ui % 2}",
                                   name=f"psw{ui % 2}")
                    box_unit(hb, cg, g_t, ps)
                    evac_unit(hb, cg, ps, sraw[0])
                for hb in range(2):
                    w32 = trans.tile([PB, C, W], f32, tag="w32")
                    iw32 = trans.tile([PB, C, W], f32, tag="big0")
                    nc.vector.tensor_copy(w32[:], sraw[0][:, hb])
                    nc.vector.reciprocal_approx_fast(out=iw32[:], in_=w32[:])
                    nc.vector.tensor_copy(sraw[1][:, hb], iw32[:])  # f32->f16
                    imb = _bcast_c(im_t[:, hb], C, bass)
                    nc.vector.tensor_mul(ivm_t[:, hb], sraw[1][:, hb], imb)
                # stage ivm row 0 for the side domain
                nc.sync.dma_start(ivm0s_t[:], ivm_t[PB - 1:PB, :, :, :])

            # ------- diffusion iterations (software-pipelined) -------
            def mktiles(t):
                p = t % 2
                e = epool.tile([PB, 2, C, WP], f16, tag=f"e{p}", name=f"e{p}")
                d01s = epool.tile([2 * C, 3, WP], f16, tag=f"d01s{p}",
                                  name=f"d01s{p}")
                eb01 = epool.tile([2 * C, 3, WP], f16, tag=f"eb01{p}",
                                  name=f"eb01{p}")
                bs01 = epool.tile([2 * C, 3, W], f16, tag=f"bs01{p}",
                                  name=f"bs01{p}")
                row0 = epool.tile([2 * C, W], f16, tag=f"row0{p}",
                                  name=f"row0{p}")
                return e, (d01s, eb01, bs01, row0)

            tiles = {0: mktiles(0)}
            for hb, cg in UNITS:
                front(hb, cg, tiles[0][0])
            side_row0(*tiles[0][1], sraw[0])
            for t in range(num_iters):
                e, _ = tiles.pop(t)
                sr = sraw[t % 2]
                last = t == num_iters - 1
                if not last:
                    tiles[t + 1] = mktiles(t + 1)
                for ui, (hb, cg) in enumerate(UNITS):
                    ps = psum.tile([PB, HC, W], f32, tag=f"ps{ui % 2}",
                                   name=f"ps{ui % 2}")
                    if ui == 0 and t > 0:
                        # p-state filler: keep the tensor engine busy across
                        # the inter-iteration gap so its clock stays ramped.
                        # Overwritten by the unit's start=True matmul below.
                        for _ in range(n_dummy):
                            nc.tensor.matmul(ps[:, 0, :], T_t[:],
                                             g_t[:, 0, 0, 1:1 + W],
                                             start=True, stop=True)
                    box_unit(hb, cg, e, ps)
                    evac_unit(hb, cg, ps, sr)
                    mul_unit(hb, cg, ui, sr)
                    if ui in (1, 3):
                        # top scheduler priority: the tree -> d -> next
                        # fronts chain is the iteration-critical tail
                        with tc.high_priority():
                            tree(hb, sr)
                            if not last:
                                front(hb, 0, tiles[t + 1][0])
                                front(hb, 1, tiles[t + 1][0])
                        if not last and ui == 3:
                            side_row0(*tiles[t + 1][1], sraw[(t + 1) % 2])

            # ---------------- output (gpsimd DMA casts f16 -> f32) --------
            nc.gpsimd.dma_start(o_d[:].rearrange("h p w -> p h w"),
                                d_t[:, :, 1:1 + W])

    nc.compile()
    return nc


def _get_built():
    global _BUILT
    if _BUILT is None:
        _BUILT = _build()
    return _BUILT


def _host_prep(guidance, blur_depth, sparse_depth):
    """Shard batch across cores; flip rows 0..127 so hb0 is row-reversed."""
    tband = np.zeros((PB, PB), np.float16)
    for k in range(PB):
        for p in range(max(0, k - 1), min(PB, k + 2)):
            tband[k, p] = 1.0
    in_maps = []
    for b in range(guidance.shape[0]):
        g = guidance[b].astype(np.float32, copy=False)
        bl = blur_depth[b, 0].astype(np.float32, copy=False)
        sp = sparse_depth[b, 0].astype(np.float32, copy=False)
        gp = np.ascontiguousarray(
            np.stack([g[:, 0:128, :], g[:, :127:-1, :]], axis=1))
        bp = np.ascontiguousarray(np.stack([bl[0:128, :], bl[:127:-1, :]]))
        spp = np.ascontiguousarray(np.stack([sp[0:128, :], sp[:127:-1, :]]))
        in_maps.append({
            "guidance": gp, "blur": bp, "sparse": spp, "tband": tband,
        })
    return in_maps


def _host_post(results):
    n = len(results)
    out = np.empty((n, 1, H, W), np.float32)
    for b in range(n):
        o = results[b]["out"]  # [2, 128, 512]
        out[b, 0, 0:PB] = o[0]
        out[b, 0, PB:] = o[1, ::-1, :]
    return out


def kernel(guidance, blur_depth, sparse_depth):
    from concourse.bass_utils import run_bass_kernel_spmd

    nc = _get_built()
    in_maps = _host_prep(guidance, blur_depth, sparse_depth)
    res = run_bass_kernel_spmd(nc, in_maps, core_ids=list(range(N_CORES)))
    return _host_post(res.results)


if __name__ == "__main__":
    rng = np.random.default_rng(0)
    g = np.abs(rng.standard_normal((B, C, H, W), dtype=np.float32))
    bl = rng.random((B, 1, H, W), dtype=np.float32)
    sp = rng.random((B, 1, H, W), dtype=np.float32)
    sp *= (rng.random((B, 1, H, W)) < 0.05)
    out = kernel(g, bl, sp)
    print(out.shape, out.dtype, np.isfinite(out).all())
